# revision 41
# baseline (speedup 1.0000x reference)
"""Distributed Trainium2 (Bass/Tile) kernel for the AdaMEOW GNN loss.

Sharding: target-node dim N row-sharded across 8 cores (128 rows each);
neighbor dim M sharded (512 each) for the neighbor-feature MLPs, combined
with one fp8 ReduceScatter (reciprocal neighbor counts are precomputed on
the host).  The h_tar/h_mask MLP runs FIRST so the y1-mean AllGather
doorbell rings as early as possible; the whole z_coarse chain (mean-adj
GCN, projection, l2-norm) is then computed FULL-N on every core inside
the ReduceScatter window - no z_coarse AllGather is needed, and the
pair-MLP B-matrix is ready before the RS lands.  Four collectives total:
AG(y1_mean), RS(neighbor agg partials), AG(fine y1), AG(fine y2), plus
the tiny attention-stats AG.  The [N,N,E] InfoNCE pair tensor is never
materialized: the pair-MLP is fused as
w[i,j] = sigmoid(sum_h tanh(A[i,h]+B[j,h])*m2[h]+b2), with sigmoid
computed via tanh to stay on one activation table; diag(logits) is
extracted with a host-provided 0/1 diagonal mask and a fused
tensor_tensor_reduce; all l2-norm rsqrts use a table-free quake-style
Newton iteration on the DVE.
"""

import os

import ml_dtypes
import numpy as np

import concourse.bass as bass
import concourse.mybir as mybir
import concourse.tile as tile
from concourse import bacc
from concourse.bass_utils import run_bass_kernel_spmd

FP = mybir.dt.float32
BF = mybir.dt.bfloat16
F8 = mybir.dt.float8e4
NPBF = ml_dtypes.bfloat16
NPF8 = ml_dtypes.float8_e4m3
AF = mybir.ActivationFunctionType
ALU = mybir.AluOpType
DR = mybir.MatmulPerfMode.DoubleRow

N, M, D0, D1, H, E = 1024, 4096, 1024, 512, 512, 64
C = 8            # cores
NL = N // C      # 128 local target nodes
ML = M // C      # 512 local neighbor nodes
P = 128
HK = H // P      # 4
D0K = D0 // P    # 8
MLK = ML // P    # 4
NB = N // P      # 8 node blocks
TAU = 0.5
RG = [list(range(C))]


def _build():
    nc = bacc.Bacc("TRN2", num_devices=C)

    def din(name, shape, dt=BF):
        return nc.declare_dram_parameter(name, list(shape), dt, isOutput=False)

    # per-core sharded inputs (host pre-arranged to final SBUF layouts)
    feat1T = din("feat1T", (P, MLK * ML), F8)     # [p, mlk, ML]
    feat2T = din("feat2T", (P, MLK * ML), F8)
    nei0T = din("nei0T", (P, MLK * N), F8)        # [p, mlk, N]
    nei1T = din("nei1T", (P, MLK * N), F8)
    recs = din("recs", (P, 2), FP)                # host 1/max(cnt,1) per view
    dmask = din("dmask", (P, N), F8)              # per-core diagonal mask
    f0m = din("f0m", (P, D0K * 2 * NL), F8)       # [p, k, tar|mask]
    adj0T = din("adj0T", (P, NB * NL), F8)
    adj1T = din("adj1T", (P, NB * NL), F8)
    madj0T = din("madj0T", (P, NB * NL), F8)
    madj1T = din("madj1T", (P, NB * NL), F8)
    mnadjTf = din("mnadjTf", (P, NB * N), F8)     # full (adj0+adj1).T
    # replicated weights
    fc0_w = din("fc0_w", (P, D0K * H), F8)
    fc1_w = din("fc1_w", (P, MLK * H), F8)
    fc2_w = din("fc2_w", (P, MLK * H), F8)
    agg0_w = din("agg0_w", (P, HK * H), F8)
    agg1_w = din("agg1_w", (P, HK * H), F8)
    gcn_w1 = din("gcn_w1", (P, HK * E), F8)
    gcn_w2 = din("gcn_w2", (E, E), BF)
    att_w = din("att_w", (E, E), BF)
    proj_w = din("proj_w", (E, E), BF)
    mlp1_w = din("mlp1_w", (E, 16), BF)
    sel8 = din("sel8", (16, P), BF)               # eye16 (x) ones(1,8)
    wm2g = din("wm2g", (P, 8 * 64), BF)           # padded m2-kron blocks
    eye128 = din("eye128", (P, P), BF)
    # small aux tensors
    fc0_b = din("fc0_b", (P, HK), FP)             # [p, hc] feature-partition
    fc1_b = din("fc1_b", (1, H), BF)              # row (for psum bias init)
    fc2_b = din("fc2_b", (1, H), BF)
    gcn_b1 = din("gcn_b1", (P, 1), FP)            # tiled x2 -> [128,1]
    gcn_b2 = din("gcn_b2", (P, 1), FP)
    attbr = din("attbr", (1, E), FP)              # att_b as a row
    projbr = din("projbr", (1, E), FP)            # proj_b as a row
    attv4 = din("attv4", (1, 4 * E), FP)          # att_vec tiled x4
    proj_b = din("proj_b", (E, 1), FP)
    mlp1_b = din("mlp1_b", (1, 16), FP)
    mlp2_b = din("mlp2_b", (1, 1), FP)

    out_ext = nc.declare_dram_parameter("out", [NL, 2], FP, isOutput=True)

    # collective bounce buffers
    agm_in = nc.dram_tensor("agm_in", [NL, E], F8)
    agm_out = nc.dram_tensor("agm_out", [N, E], F8, addr_space="Shared")
    rs_in = nc.dram_tensor("rs_in", [NB, P, 2 * HK, P], F8)
    rs_out = nc.dram_tensor("rs_out", [P, 2 * HK, P], F8)
    ag1_in = nc.dram_tensor("ag1_in", [NL, 4 * E], F8)
    ag1_out = nc.dram_tensor("ag1_out", [N, 4 * E], F8, addr_space="Shared")
    ag2_in = nc.dram_tensor("ag2_in", [NL, 4 * E], F8)
    ag2_out = nc.dram_tensor("ag2_out", [N, 4 * E], F8, addr_space="Shared")
    ag3a_in = nc.dram_tensor("ag3a_in", [1, P], BF)
    dramA = nc.dram_tensor("dramA", [NL, 16], FP)
    dramRN = nc.dram_tensor("dramRN", [1, N], BF)
    ag3a_out = nc.dram_tensor("ag3a_out", [C, P], BF, addr_space="Shared")

    with tile.TileContext(nc) as tc:
        with (
            tc.tile_pool(name="pers", bufs=1) as pers,
            tc.tile_pool(name="wkE", bufs=4) as wkE,
            tc.tile_pool(name="wkT", bufs=4) as wkT,
            tc.tile_pool(name="wkS", bufs=2) as wkS,
        ):
            def mk(pool, shape, name, dt=FP):
                return pool.tile(list(shape), dt, tag=name, name=name)

            def ld(pool, dram, shape, name, eng=None):
                t = mk(pool, shape, name, dt=dram.dtype)
                src = dram[:]
                if list(t.shape) != list(dram.shape):
                    src = src.rearrange("p (a b) -> p a b", a=t.shape[1])
                (eng or nc.sync).dma_start(t[:], src)
                return t

            def elu(ps_ap, ebias=0.0):
                """elu(x) = relu(x) + min(exp(x), 1) - 1; 2 ACT + 1 DVE."""
                sh = [ps_ap.shape[0], ps_ap.free_size()]
                e = wkE.tile(sh, BF, tag="elu_e", name="elu_e")
                r = wkE.tile(sh, BF, tag="elu_r", name="elu_r")
                nc.scalar.activation(e[:], ps_ap, AF.Exp, bias=ebias)
                nc.scalar.activation(r[:], ps_ap, AF.Relu, bias=ebias)
                q = wkE.tile(sh, BF, tag="elu_q", name="elu_q")
                nc.vector.tensor_scalar(
                    out=q[:], in0=e[:], scalar1=1.0, scalar2=-1.0,
                    op0=ALU.min, op1=ALU.add)
                return q, r

            def rsqrt_tile(ps_ap, p, f, tag, out_dt=BF):
                """Table-free rsqrt of a [p, f] psum/sbuf tile (quake
                seed + one Newton step on DVE)."""
                x = wkS.tile([p, f], FP, tag="rsq_x", name=tag + "x")
                nc.vector.tensor_scalar_max(x[:], ps_ap, 1e-24)
                sh = wkS.tile([p, f], FP, tag="rsq_s", name=tag + "s")
                nc.vector.tensor_scalar(
                    out=sh[:].bitcast(mybir.dt.uint32),
                    in0=x[:].bitcast(mybir.dt.uint32),
                    scalar1=1, scalar2=0,
                    op0=ALU.logical_shift_right, op1=ALU.bitwise_or)
                y = wkS.tile([p, f], FP, tag="rsq_y", name=tag + "y")
                nc.vector.tensor_tensor(
                    out=y[:].bitcast(mybir.dt.uint32),
                    in0=magic_sb[0:p, 0:f].bitcast(mybir.dt.uint32),
                    in1=sh[:].bitcast(mybir.dt.uint32),
                    op=ALU.subtract)
                # Newton step(s): y *= 1.5 - 0.5 x y^2
                for it in range(1):
                    t = wkS.tile([p, f], FP, tag="rsq_t", name=tag + "t")
                    nc.vector.tensor_tensor(out=t[:], in0=y[:], in1=y[:],
                                            op=ALU.mult)
                    nc.vector.tensor_tensor(out=t[:], in0=t[:], in1=x[:],
                                            op=ALU.mult)
                    nc.vector.tensor_scalar(
                        out=t[:], in0=t[:], scalar1=-0.5, scalar2=1.5,
                        op0=ALU.mult, op1=ALU.add)
                    nc.vector.tensor_tensor(out=y[:], in0=y[:], in1=t[:],
                                            op=ALU.mult)
                if out_dt == FP:
                    return y
                yb = wkS.tile([p, f], out_dt, tag="rsq_b", name=tag + "b")
                nc.vector.tensor_copy(yb[:], y[:])
                return yb

            def rsqrt_row(ps_ap, nl, tag):
                return rsqrt_tile(ps_ap, 1, nl, tag)

            # ---------------- persistent constants --------------------
            ones_row = mk(pers, (1, 512), "ones_row", BF)
            nc.vector.memset(ones_row[:], 1.0)
            ones_col = mk(pers, (P, 1), "ones_col", BF)
            nc.vector.memset(ones_col[:], 1.0)
            magic_sb = mk(pers, (P, 512), "magic", FP)
            nc.vector.memset(magic_sb[:].bitcast(mybir.dt.uint32),
                             0x5f3759df)

            def ld2(pool, dram, shape, name, eng0, eng1):
                """Split a [P, a, b] load into two halves on two engine
                queues so the DMA rings drain it in parallel."""
                t = mk(pool, shape, name, dt=dram.dtype)
                src = dram[:].rearrange("p (a b) -> p a b", a=shape[1])
                h = shape[1] // 2
                eng0.dma_start(t[:, 0:h, :], src[:, 0:h, :])
                eng1.dma_start(t[:, h:, :], src[:, h:, :])
                return t

            # ================= stage 1: fp8 MLPs + aggregation ========
            f0m_sb = ld(pers, f0m, (P, D0K, 2 * NL), "f0m", nc.sync)
            fc0w_sb = ld(pers, fc0_w, (P, D0K, H), "fc0w", nc.gpsimd)
            fc0b_sb = ld(pers, fc0_b, (P, HK), "fc0b", nc.scalar)
            gcnw1_sb = ld(pers, gcn_w1, (P, HK, E), "gcnw1", nc.scalar)
            feat1T_sb = ld(pers, feat1T, (P, MLK, ML), "feat1T", nc.gpsimd)
            fc1w_sb = ld(pers, fc1_w, (P, MLK, H), "fc1w", nc.sync)
            feat2T_sb = ld(pers, feat2T, (P, MLK, ML), "feat2T", nc.gpsimd)
            fc2w_sb = ld(pers, fc2_w, (P, MLK, H), "fc2w", nc.sync)
            nei0T_sb = ld(pers, nei0T, (P, MLK, N), "nei0T", nc.gpsimd)
            nei1T_sb = ld(pers, nei1T, (P, MLK, N), "nei1T", nc.sync)
            fc1b_sb = ld(pers, fc1_b, (1, H), "fc1b", nc.scalar)
            fc2b_sb = ld(pers, fc2_b, (1, H), "fc2b", nc.scalar)

            hnei_sb = [mk(pers, (P, MLK, H), "hnei0", F8),
                       mk(pers, (P, MLK, H), "hnei1", F8)]

            with tc.tile_pool(name="psA", bufs=3, space="PSUM") as psA:
                # ---- h[tar|mask]T + y1_mean FIRST: rings the y1-mean
                # AllGather doorbell as early as possible so the AG mesh
                # completes before rs_in is ready (un-gates the RS) ----
                hthm_sb = mk(pers, (P, HK, 4 * NL), "hthm", BF)
                for hc in range(HK):
                    ps = psA.tile([P, 2 * NL], FP, tag="psA", name="ps_ht")
                    for kp in range(D0K // 2):
                        nc.tensor.matmul(
                            ps[:],
                            fc0w_sb[:, 2 * kp:2 * kp + 2,
                                    hc * P:(hc + 1) * P],
                            f0m_sb[:, 2 * kp:2 * kp + 2, :],
                            start=(kp == 0), stop=(kp == D0K // 2 - 1),
                            perf_mode=DR)
                    q, r = elu(ps[:], ebias=fc0b_sb[:, hc:hc + 1])
                    nc.vector.tensor_tensor(
                        out=hthm_sb[:, hc, 0:2 * NL], in0=q[:], in1=r[:],
                        op=ALU.add)
                htar8 = mk(pers, (P, HK, NL), "htar8", F8)
                nc.vector.tensor_copy(htar8[:], hthm_sb[:, :, 0:NL])
                psm = psA.tile([P, E], FP, tag="psA", name="ps_y1m")
                for kp in range(HK // 2):
                    nc.tensor.matmul(
                        psm[:], htar8[:, 2 * kp:2 * kp + 2, :],
                        gcnw1_sb[:, 2 * kp:2 * kp + 2, :],
                        start=(kp == 0), stop=(kp == HK // 2 - 1),
                        perf_mode=DR)
                stm = wkS.tile([NL, E], F8, tag="stm", name="stm")
                nc.vector.tensor_scalar_mul(stm[:], psm[:], 0.5)
                nc.sync.dma_start(agm_in[:], stm[:])

                # ---- h_nei shards: elu(featX @ fcX_w + b) in fp8 -----
                for v, (fT, fw, fb) in enumerate(
                    [(feat1T_sb, fc1w_sb, fc1b_sb),
                     (feat2T_sb, fc2w_sb, fc2b_sb)]
                ):
                    for mc in range(MLK):
                        ps = psA.tile([P, H], FP, tag="psA", name="ps_hnei")
                        nc.tensor.matmul(ps[:], ones_row[:, 0:P], fb[:],
                                         start=True, stop=False)
                        for kp in range(MLK // 2):
                            nc.tensor.matmul(
                                ps[:],
                                fT[:, 2 * kp:2 * kp + 2, mc * P:(mc + 1) * P],
                                fw[:, 2 * kp:2 * kp + 2, :],
                                start=False, stop=(kp == MLK // 2 - 1),
                                perf_mode=DR)
                        q, r = elu(ps[:])
                        nc.vector.tensor_tensor(
                            out=hnei_sb[v][:, mc, :], in0=q[:], in1=r[:],
                            op=ALU.add)

                for hc in range(HK):
                    nc.vector.tensor_copy(hthm_sb[:, hc, 2 * NL:4 * NL],
                                          hthm_sb[:, hc, 0:2 * NL])

                # ---- partial aggregation (feature-major, fp8) --------
                wq = [nc.sync, nc.scalar]
                for v, neiT in enumerate([nei0T_sb, nei1T_sb]):
                    for hc in range(HK):
                        for jh in range(2):
                            ps = psA.tile([P, 512], FP, tag="psA",
                                          name="ps_pr")
                            for kp in range(MLK // 2):
                                nc.tensor.matmul(
                                    ps[:],
                                    hnei_sb[v][:, 2 * kp:2 * kp + 2,
                                               hc * P:(hc + 1) * P],
                                    neiT[:, 2 * kp:2 * kp + 2,
                                         jh * 512:(jh + 1) * 512],
                                    start=(kp == 0),
                                    stop=(kp == MLK // 2 - 1),
                                    perf_mode=DR)
                            prs = wkE.tile([P, 512], F8, tag="prs",
                                           name="prs")
                            if (hc + jh) % 2 == 0:
                                nc.vector.tensor_copy(prs[:], ps[:])
                            else:
                                nc.scalar.activation(prs[:], ps[:],
                                                     AF.Copy)
                            wq[(hc + jh) % 2].dma_start(
                                rs_in[jh * 4:(jh + 1) * 4, :,
                                      v * HK + hc:v * HK + hc + 1,
                                      :].rearrange(
                                          "b p one n -> p b (one n)"),
                                prs[:].rearrange("p (b n) -> p b n", b=4))

            # cc stream order: AG(y1_mean) -> RS -> AG(zc) -> AG1 ...
            nc.gpsimd.collective_compute(
                "AllGather", ALU.bypass, replica_groups=RG,
                ins=[agm_in[:].opt()], outs=[agm_out[:].opt()])
            nc.gpsimd.collective_compute(
                "ReduceScatter", ALU.add, replica_groups=RG,
                ins=[rs_in[:].opt()], outs=[rs_out[:].opt()])

            # ============ phase 1 (overlaps the ReduceScatter) ========
            with tc.tile_pool(name="psB", bufs=3, space="PSUM") as psB, \
                 tc.tile_pool(name="psS", bufs=4, space="PSUM") as psS:
                adj0T_sb = ld(pers, adj0T, (P, NB, NL), "adj0T",
                              nc.gpsimd)
                adj1T_sb = ld(pers, adj1T, (P, NB, NL), "adj1T",
                              nc.gpsimd)
                madj0T_sb = ld(pers, madj0T, (P, NB, NL), "madj0T",
                               nc.gpsimd)
                madj1T_sb = ld(pers, madj1T, (P, NB, NL), "madj1T",
                               nc.gpsimd)
                mnadjTf_sb = ld(pers, mnadjTf, (P, NB, N), "mnadjTf",
                                nc.gpsimd)
                agg0w_sb = ld(pers, agg0_w, (P, HK, H), "agg0w", nc.sync)
                agg1w_sb = ld(pers, agg1_w, (P, HK, H), "agg1w", nc.scalar)
                recs_sb = ld(pers, recs, (P, 2), "recs", nc.sync)
                dmask_sb = ld(pers, dmask, (P, N), "dmask", nc.gpsimd)
                gcnw2_sb = ld(pers, gcn_w2, (E, E), "gcnw2", nc.sync)
                attw_sb = ld(pers, att_w, (E, E), "attw", nc.sync)
                projw_sb = ld(pers, proj_w, (E, E), "projw", nc.sync)
                mlp1w_sb = ld(pers, mlp1_w, (E, 16), "mlp1w", nc.sync)
                sel8_sb = ld(pers, sel8, (16, P), "sel8", nc.scalar)
                wm2g_sb = ld(pers, wm2g, (P, 8, 64), "wm2g", nc.scalar)
                eye_sb = ld(pers, eye128, (P, P), "eye", nc.scalar)
                gcnb1_sb = ld(pers, gcn_b1, (P, 1), "gcnb1", nc.sync)
                gcnb2_sb = ld(pers, gcn_b2, (P, 1), "gcnb2", nc.sync)
                projb_sb = ld(pers, proj_b, (E, 1), "projb", nc.sync)
                attb_bcT = mk(pers, (P, E), "attb_bcT")
                nc.sync.dma_start(attb_bcT[:], attbr[:].to_broadcast((P, E)))
                projb_bcT = mk(pers, (P, E), "projb_bcT")
                nc.sync.dma_start(projb_bcT[:],
                                  projbr[:].to_broadcast((P, E)))
                attv4_sb = ld(pers, attv4, (1, 4 * E), "attv4", nc.scalar)
                b1bc16 = mk(pers, (P, 16), "b1bc16")
                nc.sync.dma_start(b1bc16[:], mlp1_b[:].to_broadcast((P, 16)))

                b2h = mk(pers, (P, 1), "b2h")
                nc.sync.dma_start(b2h[:], mlp2_b[:].to_broadcast((P, 1)))
                nc.vector.tensor_scalar_mul(b2h[:], b2h[:], 0.5)

                # reciprocal counts precomputed on host (recs input)
                rec4 = []
                for v in range(2):
                    rcb = wkS.tile([P, 1], BF, tag="rcb", name="rcb")
                    nc.vector.tensor_copy(rcb[:], recs_sb[:, v:v + 1])
                    pst = psS.tile([1, P], FP, tag="psS", name="ps_rT")
                    nc.tensor.matmul(pst[:], rcb[:], eye_sb[:])
                    rrow4 = wkS.tile([1, 4, P], BF, tag="rrow4",
                                     name="rrow4")
                    for t4 in range(4):
                        nc.vector.tensor_copy(rrow4[:, t4, :], pst[:])
                    psb = psB.tile([P, 4 * P], FP, tag="psB", name="ps_rbc")
                    nc.tensor.matmul(psb[:], ones_row[:, 0:P],
                                     rrow4[:].rearrange("o a b -> o (a b)"))
                    rb = mk(pers, (P, 4 * P), f"rec4_{v}", BF)
                    nc.vector.tensor_copy(rb[:], psb[:])
                    rec4.append(rb)

                # ---- z_coarse chain: full-N mean conv (in RS window) -
                y1m_sb = mk(pers, (P, NB, E), "y1mall", F8)
                nc.sync.dma_start(
                    y1m_sb[:], agm_out[:].rearrange("(b p) e -> p b e", p=P))
                hmT_sb = mk(pers, (E, N), "hmT", F8)
                for jh in range(2):
                    ps = psB.tile([P, 512], FP, tag="psB", name="ps_hm")
                    for bp in range(NB // 2):
                        nc.tensor.matmul(
                            ps[0:E, :], y1m_sb[:, 2 * bp:2 * bp + 2, :],
                            mnadjTf_sb[:, 2 * bp:2 * bp + 2,
                                       jh * 512:(jh + 1) * 512],
                            start=(bp == 0), stop=(bp == NB // 2 - 1),
                            perf_mode=DR)
                    nc.vector.tensor_scalar(
                        out=hmT_sb[:, jh * 512:(jh + 1) * 512],
                        in0=ps[0:E, :], scalar1=gcnb1_sb[0:E, :],
                        scalar2=0.0, op0=ALU.add, op1=ALU.max)
                # y2_mean (x0.5 for the mean-adj sum) [p, NB, E] fp8
                y2m_sb = mk(pers, (P, NB, E), "y2m", F8)
                for b in range(NB):
                    ps = psS.tile([P, E], FP, tag="psS", name="ps_y2m")
                    nc.tensor.matmul(ps[:], hmT_sb[:, b * P:(b + 1) * P],
                                     gcnw2_sb[:])
                    nc.vector.tensor_scalar_mul(y2m_sb[:, b, :], ps[:], 0.5)
                # ---- z_coarse FULL-N (replicated on every core; no
                # AllGather needed): conv with full mean-adj, then
                # proj+tanh+colnorm per 512-col half -> zcall [E, N] ---
                zT_sb = mk(pers, (E, 4, NL), "zT", BF)
                zcall_sb = mk(pers, (E, N), "zcall", BF)
                BT_sb = mk(pers, (16, N), "BT", BF)
                tfcall = mk(pers, (E, N), "tfcall", BF)
                for jh in range(2):
                    pszm = psB.tile([P, 512], FP, tag="psB", name="ps_zm")
                    for bp in range(NB // 2):
                        nc.tensor.matmul(
                            pszm[0:E, :], y2m_sb[:, 2 * bp:2 * bp + 2, :],
                            mnadjTf_sb[:, 2 * bp:2 * bp + 2,
                                       jh * 512:(jh + 1) * 512],
                            start=(bp == 0), stop=(bp == NB // 2 - 1),
                            perf_mode=DR)
                    zcf = wkS.tile([E, 512], BF, tag="zcf", name="zcf")
                    nc.vector.tensor_scalar_add(zcf[:], pszm[0:E, :],
                                                gcnb2_sb[0:E, :])
                    psz = psB.tile([P, 512], FP, tag="psB", name="ps_pzc")
                    nc.tensor.matmul(psz[0:E, :], projw_sb[:], zcf[:])
                    nc.scalar.activation(
                        tfcall[:, jh * 512:(jh + 1) * 512], psz[0:E, :],
                        AF.Tanh, bias=projb_sb[:])
                # node-major norms: transpose 128-col blocks, square on
                # ACT, row-reduce -> [P, NB]; rsqrt with free-dim 8 is
                # ~100x cheaper than the [1, N] row variant on the DVE
                nzc = wkS.tile([P, NB], FP, tag="nzc", name="nzc")
                for q in range(4):
                    tq = psS.tile([P, 2, E], FP, tag="psS", name="ps_tq")
                    for s in range(2):
                        b = 2 * q + s
                        nc.tensor.matmul(tq[:, s, :],
                                         tfcall[:, b * P:(b + 1) * P],
                                         eye_sb[0:E, 0:E])
                    sqb = wkE.tile([P, 2, E], BF, tag="sqv", name="sqb")
                    nc.scalar.activation(
                        sqb[:].rearrange("p a b -> p (a b)"),
                        tq[:].rearrange("p a b -> p (a b)"), AF.Square)
                    nc.vector.reduce_sum(nzc[:, 2 * q:2 * q + 2], sqb[:],
                                         axis=mybir.AxisListType.X)
                rz8 = rsqrt_tile(nzc[:], P, NB, "rz8")
                nc.sync.dma_start(
                    dramRN[:].rearrange("o (b n) -> n (o b)", n=P), rz8[:])
                rnrow = wkS.tile([1, N], BF, tag="rnrow", name="rnrow")
                nc.sync.dma_start(rnrow[:], dramRN[:])
                for jh in range(2):
                    hf = slice(jh * 512, (jh + 1) * 512)
                    psbz = psB.tile([P, 512], FP, tag="psB", name="ps_nbz")
                    nc.tensor.matmul(psbz[0:E, :], ones_row[:, 0:E],
                                     rnrow[:, hf])
                    nc.vector.tensor_mul(zcall_sb[:, hf], tfcall[:, hf],
                                         psbz[0:E, :])
                    pbt = psB.tile([P, 512], FP, tag="psB", name="ps_BT")
                    nc.tensor.matmul(pbt[0:16, :], mlp1w_sb[:],
                                     zcall_sb[:, hf])
                    nc.vector.tensor_copy(BT_sb[:, hf], pbt[0:16, :])

                # ================= post-RS: views + fine GCN ==========
                aggT_sb = mk(pers, (P, 2 * HK, NL), "aggT", F8)
                nc.sync.dma_start(aggT_sb[:, 0:HK, :], rs_out[:, 0:HK, :])
                nc.scalar.dma_start(aggT_sb[:, HK:, :], rs_out[:, HK:, :])
                aggS_sb = mk(pers, (P, 2 * HK, 2 * NL), "aggS", F8)
                for v in range(2):
                    for half in range(2):
                        nc.vector.tensor_tensor(
                            out=aggS_sb[:, v * HK:(v + 1) * HK,
                                        half * NL:(half + 1) * NL],
                            in0=aggT_sb[:, v * HK:(v + 1) * HK, :],
                            in1=rec4[v][:].rearrange(
                                "p (a b) -> p a b", a=HK),
                            op=ALU.mult)

                # both views + masks in one [P, 512] pass per h-chunk:
                # cols [v0tar | v0mask | v1tar | v1mask]
                xs4 = mk(pers, (P, HK, 4 * NL), "xs4", F8)
                for hc in range(HK):
                    ps = psB.tile([P, 4 * NL], FP, tag="psB", name="ps_x2")
                    for v, aggw in enumerate([agg0w_sb, agg1w_sb]):
                        half = ps[:, v * 2 * NL:(v + 1) * 2 * NL]
                        nc.tensor.matmul(
                            half, eye_sb[:],
                            hthm_sb[:, hc, v * 2 * NL:(v + 1) * 2 * NL],
                            start=True, stop=False)
                        for kp in range(HK // 2):
                            nc.tensor.matmul(
                                half,
                                aggw[:, 2 * kp:2 * kp + 2,
                                     hc * P:(hc + 1) * P],
                                aggS_sb[:, v * HK + 2 * kp:
                                        v * HK + 2 * kp + 2, :],
                                start=False, stop=(kp == HK // 2 - 1),
                                perf_mode=DR)
                    q, r = elu(ps[:])
                    nc.vector.tensor_tensor(
                        out=xs4[:, hc, :], in0=q[:], in1=r[:], op=ALU.add)

                # GCN layer-1 linear; st4a cols [v0, v1, m0, m1]
                st4a = mk(pers, (NL, 4, E), "st4a", F8)
                for c0, slot in [(0, 0), (2 * NL, 1), (NL, 2), (3 * NL, 3)]:
                    ps = psS.tile([P, E], FP, tag="psS", name="ps_y1")
                    for kp in range(HK // 2):
                        nc.tensor.matmul(
                            ps[:], xs4[:, 2 * kp:2 * kp + 2, c0:c0 + NL],
                            gcnw1_sb[:, 2 * kp:2 * kp + 2, :],
                            start=(kp == 0), stop=(kp == HK // 2 - 1),
                            perf_mode=DR)
                    nc.vector.tensor_copy(st4a[:, slot, :], ps[:])
                nc.sync.dma_start(
                    ag1_in[:].rearrange("n (g e) -> n g e", g=4), st4a[:])
                nc.gpsimd.collective_compute(
                    "AllGather", ALU.bypass, replica_groups=RG,
                    ins=[ag1_in[:].opt()], outs=[ag1_out[:].opt()])

                def conv_fine(y_sb, badd, relu, outs):
                    """4 fine graph convs; y_sb [P, NB, 4E] fp8 cols
                    [v0, v1, m0, m1]; outs: list of 4 (dst_ap)."""
                    pp = [psS.tile([E, NL], FP, tag="psS", name=f"pc{g}")
                          for g in range(4)]
                    adjs = [adj0T_sb, adj1T_sb, madj0T_sb, madj1T_sb]
                    for bp in range(NB // 2):
                        for g in range(4):
                            nc.tensor.matmul(
                                pp[g][:],
                                y_sb[:, 2 * bp:2 * bp + 2,
                                     g * E:(g + 1) * E],
                                adjs[g][:, 2 * bp:2 * bp + 2, :],
                                start=(bp == 0), stop=(bp == NB // 2 - 1),
                                perf_mode=DR)
                    op1 = ALU.max if relu else ALU.bypass
                    for g in range(4):
                        nc.vector.tensor_scalar(
                            out=outs[g], in0=pp[g][:],
                            scalar1=badd[0:E, :], scalar2=0.0,
                            op0=ALU.add, op1=op1)

                y1_sb = mk(pers, (P, NB, 4 * E), "y1", F8)
                y1src = ag1_out[:].rearrange("(b p) f -> p b f", p=P)
                for qi, qe in enumerate([nc.sync, nc.scalar,
                                         nc.sync, nc.scalar]):
                    qe.dma_start(y1_sb[:, 2 * qi:2 * qi + 2, :],
                                 y1src[:, 2 * qi:2 * qi + 2, :])
                h4_sb = mk(pers, (E, 4, NL), "h4", BF)
                conv_fine(y1_sb, gcnb1_sb, True,
                          [h4_sb[:, g, :] for g in range(4)])
                st4b = mk(pers, (NL, 4, E), "st4b", F8)
                for g in range(4):
                    ps = psS.tile([P, E], FP, tag="psS", name="ps_y2")
                    nc.tensor.matmul(ps[:], h4_sb[:, g, :], gcnw2_sb[:])
                    nc.vector.tensor_copy(st4b[:, g, :], ps[:])
                nc.sync.dma_start(
                    ag2_in[:].rearrange("n (g e) -> n g e", g=4), st4b[:])
                nc.gpsimd.collective_compute(
                    "AllGather", ALU.bypass, replica_groups=RG,
                    ins=[ag2_in[:].opt()], outs=[ag2_out[:].opt()])

                y2_sb = mk(pers, (P, NB, 4 * E), "y2", F8)
                y2src = ag2_out[:].rearrange("(b p) f -> p b f", p=P)
                for qi, qe in enumerate([nc.sync, nc.scalar,
                                         nc.sync, nc.scalar]):
                    qe.dma_start(y2_sb[:, 2 * qi:2 * qi + 2, :],
                                 y2src[:, 2 * qi:2 * qi + 2, :])
                # conv2 -> zT slots [v0, v1, m0, m1] -> [v0, m0, v1, m1]
                conv_fine(y2_sb, gcnb2_sb, False,
                          [zT_sb[:, 0, :], zT_sb[:, 2, :],
                           zT_sb[:, 1, :], zT_sb[:, 3, :]])

                # ---- attention, node-major: every per-node scalar
                # (norm, beta-weight) lives on partitions so the rsqrt
                # and scale ops run full-lane instead of on one row ----
                attT = psS.tile([P, 4, E], FP, tag="psS", name="ps_attT")
                z4T = psS.tile([P, 4, E], FP, tag="psS", name="ps_z4T")
                pj4T = psS.tile([P, 4, E], FP, tag="psS", name="ps_pj4T")
                for v in range(4):
                    zv = zT_sb[:, v, :]
                    nc.tensor.matmul(attT[:, v, :], zv, attw_sb[:])
                    nc.tensor.matmul(z4T[:, v, :], zv,
                                     eye_sb[0:E, 0:E])
                    nc.tensor.matmul(pj4T[:, v, :], zv, projw_sb[:])
                norm4 = wkS.tile([P, 4], FP, tag="norm4", name="norm4")
                sq4T = wkS.tile([P, 4, E], BF, tag="sq4T", name="sq4T")
                nc.scalar.activation(
                    sq4T[:].rearrange("p a b -> p (a b)"),
                    z4T[:].rearrange("p a b -> p (a b)"), AF.Square)
                nc.vector.reduce_sum(norm4[:], sq4T[:],
                                     axis=mybir.AxisListType.X)
                rn4T = rsqrt_tile(norm4[:], P, 4, "rn4", out_dt=FP)
                taT = wkS.tile([P, 4, E], BF, tag="taT", name="taT")
                for v in range(4):
                    nc.vector.scalar_tensor_tensor(
                        out=taT[:, v, :], in0=attT[:, v, :],
                        scalar=rn4T[:, v:v + 1], in1=attb_bcT[:],
                        op0=ALU.mult, op1=ALU.add)
                taTt = wkS.tile([P, 4, E], BF, tag="taTt", name="taTt")
                nc.scalar.activation(
                    taTt[:].rearrange("p a b -> p (a b)"),
                    taT[:].rearrange("p a b -> p (a b)"), AF.Tanh)
                psE4 = psS.tile([1, 4, E], FP, tag="psSt",
                                name="ps_e4", bufs=1)
                for v in range(4):
                    nc.tensor.matmul(psE4[:, v, :], ones_col[:],
                                     taTt[:, v, :])
                se4 = wkS.tile([1, 4, E], FP, tag="se4", name="se4")
                nc.vector.tensor_mul(
                    se4[:].rearrange("o a b -> o (a b)"),
                    psE4[:].rearrange("o a b -> o (a b)"), attv4_sb[:])
                er4 = wkS.tile([1, 4], FP, tag="er4", name="er4")
                nc.vector.reduce_sum(er4[:], se4[:],
                                     axis=mybir.AxisListType.X)
                e_row = wkS.tile([1, P], BF, tag="e_row", name="e_row")
                nc.vector.memset(e_row[:], 0.0)
                nc.vector.tensor_scalar_mul(e_row[:, 0:4], er4[:], 1.0 / N)
                nc.sync.dma_start(ag3a_in[:], e_row[:])
                nc.gpsimd.collective_compute(
                    "AllGather", ALU.bypass, replica_groups=RG,
                    ins=[ag3a_in[:].opt()], outs=[ag3a_out[:].opt()])

                # ---- softmax over views; z_fine (node-major); A ------
                e8_sb = wkS.tile([C, 4], BF, tag="e8", name="e8")
                nc.sync.dma_start(e8_sb[:], ag3a_out[:, 0:4])
                pse2 = psS.tile([1, 4], FP, tag="psSt", name="ps_e2",
                                bufs=1)
                nc.tensor.matmul(pse2[:], ones_col[0:C, :], e8_sb[:])
                ee = wkS.tile([1, 4], FP, tag="ee", name="ee")
                nc.scalar.activation(ee[:], pse2[:], AF.Exp)
                se = wkS.tile([1, 1], FP, tag="se", name="se")
                nc.vector.reduce_sum(se[:], ee[:], axis=mybir.AxisListType.X)
                nc.vector.reciprocal(se[:], se[:])
                beta_row = wkS.tile([1, 4], BF, tag="beta", name="beta")
                nc.vector.tensor_scalar_mul(beta_row[:], ee[:], se[:])
                psbb = psS.tile([P, 4], FP, tag="psSt",
                                name="ps_beta", bufs=1)
                nc.tensor.matmul(psbb[:], ones_row[:, 0:P], beta_row[:])
                rnb4 = wkS.tile([P, 4], FP, tag="rnb4", name="rnb4")
                nc.vector.tensor_mul(rnb4[:], rn4T[:], psbb[:])
                zfpT = wkS.tile([P, E], FP, tag="zfpT", name="zfpT")
                nc.vector.tensor_scalar(
                    out=zfpT[:], in0=pj4T[:, 0, :], scalar1=rnb4[:, 0:1],
                    scalar2=0.0, op0=ALU.mult, op1=ALU.add)
                for v in range(1, 4):
                    nc.vector.scalar_tensor_tensor(
                        out=zfpT[:], in0=pj4T[:, v, :],
                        scalar=rnb4[:, v:v + 1], in1=zfpT[:],
                        op0=ALU.mult, op1=ALU.add)
                tf2T = wkS.tile([P, E], BF, tag="tf2T", name="tf2T")
                nc.vector.tensor_add(zfpT[:], zfpT[:], projb_bcT[:])
                nc.scalar.activation(tf2T[:], zfpT[:], AF.Tanh)
                # 1/||zf|| per anchor, already on partitions
                nrm1 = wkS.tile([P, 1], FP, tag="nrm1", name="nrm1")
                sqf = wkE.tile([P, E], BF, tag="sqv", name="sqf")
                nc.vector.scalar_tensor_tensor(
                    out=sqf[:], in0=tf2T[:], scalar=1.0, in1=tf2T[:],
                    op0=ALU.bypass, op1=ALU.mult, accum_out=nrm1[:])
                recn = rsqrt_tile(nrm1[:], P, 1, "rzf", out_dt=FP)
                rec2 = mk(pers, (P, 1), "rec2")
                nc.vector.tensor_scalar_mul(rec2[:], recn[:], 1.0 / TAU)
                # transpose tf2T back to [E, NL] for the logits matmuls
                tf2ps = psS.tile([E, NL], FP, tag="psSt",
                                 name="ps_tf2", bufs=1)
                nc.tensor.matmul(tf2ps[:], tf2T[:], eye_sb[:])
                tf2 = mk(pers, (E, NL), "tf2", BF)
                nc.vector.tensor_copy(tf2[:], tf2ps[:])
                psa2 = psS.tile([NL, 16], FP, tag="psSt",
                                name="ps_A", bufs=1)
                nc.tensor.matmul(psa2[:], tf2[:], mlp1w_sb[:])
                A_sb = mk(pers, (NL, 16), "A")
                nc.vector.scalar_tensor_tensor(
                    out=A_sb[:], in0=psa2[:], scalar=recn[:, 0:1],
                    in1=b1bc16[:], op0=ALU.mult, op1=ALU.add)
                # rearrange A[i,h] -> A_all[(h*8+k), ic] (i = ic*8+k) via
                # a DRAM round-trip so the pair-MLP h-sum becomes a PE
                # contraction instead of 16 DVE multiply-adds
                nc.sync.dma_start(dramA[:], A_sb[:])
                A_all = mk(pers, (P, 16), "A_all")
                nc.sync.dma_start(
                    A_all[:],
                    dramA[:].rearrange("(ic k) h -> (k h) ic", k=8))

            # ================= InfoNCE tail (2-bank psum) =============
            with tc.tile_pool(name="psT", bufs=3, space="PSUM") as psT:
                # dots = exp((zfn.zc) / tau)  (exp/tanh table)
                psl = psT.tile([P, N], FP, tag="psT", name="ps_log")
                for jh in range(2):
                    nc.tensor.matmul(
                        psl[:, jh * 512:(jh + 1) * 512], tf2[:],
                        zcall_sb[:, jh * 512:(jh + 1) * 512])
                dots_sb = mk(pers, (P, N), "dots", BF)
                nc.scalar.activation(dots_sb[:], psl[:], AF.Exp,
                                     scale=rec2[:, 0:1])
                s0_sb = wkS.tile([P, 1], FP, tag="s0", name="s0")
                nc.vector.reduce_sum(s0_sb[:], dots_sb[:],
                                     axis=mybir.AxisListType.X)
                # diag(logits): mask out the per-core diagonal of the
                # raw logit block and row-reduce; scaled by rec2 at the
                # end (replaces the separate local-zc dot product)
                scrd = wkT.tile([P, N], BF, tag="th", name="scrd")
                dgr = wkS.tile([P, 1], FP, tag="dgr", name="dgr")
                nc.vector.scalar_tensor_tensor(
                    out=scrd[:], in0=psl[:], scalar=1.0, in1=dmask_sb[:],
                    op0=ALU.bypass, op1=ALU.mult, accum_out=dgr[:])

                # acc[i,j] = sum_h tanh(A[i,h] + B[j,h]) * m2[h]:
                # partitions hold (h*8+k) for an 8-anchor chunk; ONE
                # B-broadcast psum serves all 16 chunks, each chunk is
                # one tanh (bias = A_all column) + a PE contraction over
                # the 16 h-lanes via the block-diagonal wm2, and the
                # [8, N] results DMA back to anchor-major SBUF rows.
                psb8 = psT.tile([P, N], FP, tag="psT", name="ps_b8")
                for jh in range(2):
                    nc.tensor.matmul(
                        psb8[:, jh * 512:(jh + 1) * 512], sel8_sb[:],
                        BT_sb[:, jh * 512:(jh + 1) * 512])
                sb8 = mk(pers, (P, N), "sb8", BF)
                nc.vector.tensor_copy(sb8[:], psb8[:])
                psaccB = psT.tile([P, N], FP, tag="psT", name="ps_accB")
                for ic in range(16):
                    g, j = divmod(ic, 8)
                    th = wkT.tile([P, N], BF, tag="th", name="th")
                    nc.scalar.activation(th[:], sb8[:], AF.Tanh,
                                         bias=A_all[:, ic:ic + 1])
                    for jh in range(2):
                        nc.tensor.matmul(
                            psaccB[64 * g:64 * (g + 1),
                                   jh * 512:(jh + 1) * 512],
                            wm2g_sb[:, j, :],
                            th[:, jh * 512:(jh + 1) * 512],
                            start=(j == 0), stop=(j == 7))

                dots_sb = mk(pers, (P, N), "dots", BF)
                nc.scalar.activation(dots_sb[:], psl[:], AF.Exp,
                                     scale=rec2[:, 0:1])
                s0_sb = wkS.tile([P, 1], FP, tag="s0", name="s0")
                nc.vector.reduce_sum(s0_sb[:], dots_sb[:],
                                     axis=mybir.AxisListType.X)
                scrd = wkT.tile([P, N], BF, tag="th", name="scrd")
                dgr = wkS.tile([P, 1], FP, tag="dgr", name="dgr")
                nc.vector.scalar_tensor_tensor(
                    out=scrd[:], in0=psl[:], scalar=1.0, in1=dmask_sb[:],
                    op0=ALU.bypass, op1=ALU.mult, accum_out=dgr[:])

                # sigmoid(x) = 0.5 + 0.5*tanh(x/2): w = 0.5*(1 + wt), so
                # denom = 0.5*(S0 + sum_j dots*wt); the 0.5 is folded into
                # the host-side log (log(x/2) = log(x) - log 2)
                wt_sb = wkT.tile([P, N], BF, tag="wt", name="wt")
                nc.scalar.activation(wt_sb[:], psaccB[:], AF.Tanh,
                                     scale=0.5, bias=b2h[:])
                d1_sb = wkS.tile([P, 1], FP, tag="denom", name="denom")
                scr = wkT.tile([P, N], BF, tag="scr", name="scr")
                nc.vector.scalar_tensor_tensor(
                    out=scr[:], in0=dots_sb[:], scalar=1.0, in1=wt_sb[:],
                    op0=ALU.bypass, op1=ALU.mult, accum_out=d1_sb[:])

                outt = wkS.tile([NL, 2], FP, tag="outt", name="outt")
                nc.vector.tensor_add(outt[:, 0:1], s0_sb[:], d1_sb[:])
                nc.vector.tensor_scalar_mul(outt[:, 1:2], dgr[:],
                                            rec2[:, 0:1])
                nc.sync.dma_start(out_ext[:], outt[:])

    nc.finalize()
    return nc


_NC_CACHE = {}


def _get_nc():
    if "nc" not in _NC_CACHE:
        _NC_CACHE["nc"] = _build()
    return _NC_CACHE["nc"]


def _part3(x, p=128):
    """[(o p), f] row-major -> [p, o*f] (partition-inner layout)."""
    o = x.shape[0] // p
    return x.reshape(o, p, x.shape[1]).transpose(1, 0, 2).reshape(p, -1)


def kernel(**inputs):
    inp = {k: np.ascontiguousarray(np.asarray(v, dtype=np.float32))
           for k, v in inputs.items()}
    nc = _get_nc()

    def bf(x):
        return np.ascontiguousarray(x.astype(NPBF))

    def f8(x):
        return np.ascontiguousarray(x.astype(NPF8))

    rep = {}
    rep["fc0_w"] = f8(_part3(inp["fc0_w"]))
    rep["fc1_w"] = f8(_part3(inp["fc1_w"]))
    rep["fc2_w"] = f8(_part3(inp["fc2_w"]))
    rep["agg0_w"] = f8(_part3(inp["agg0_w"]))
    rep["agg1_w"] = f8(_part3(inp["agg1_w"]))
    rep["gcn_w1"] = f8(_part3(inp["gcn_w1"]))
    for k in ["gcn_w2", "att_w", "proj_w", "mlp1_w"]:
        rep[k] = bf(inp[k])
    rep["sel8"] = bf(
        np.kron(np.ones((1, 8), np.float32), np.eye(16, dtype=np.float32)))
    _w8 = np.kron(np.eye(8, dtype=np.float32),
                  inp["mlp2_w"].reshape(16, 1).astype(np.float32))
    _wg = np.zeros((P, 8, 64), np.float32)
    for _j in range(8):
        _wg[:, _j, 8 * _j:8 * (_j + 1)] = _w8
    rep["wm2g"] = bf(_wg.reshape(P, 8 * 64))
    rep["eye128"] = bf(np.eye(P, dtype=np.float32))
    rep["fc0_b"] = np.ascontiguousarray(
        inp["fc0_b"].reshape(HK, P).T)                     # [p, hc]
    rep["fc1_b"] = bf(inp["fc1_b"].reshape(1, H))
    rep["fc2_b"] = bf(inp["fc2_b"].reshape(1, H))
    rep["gcn_b1"] = np.ascontiguousarray(
        np.tile(inp["gcn_b1"].reshape(E), 2).reshape(P, 1))
    rep["gcn_b2"] = np.ascontiguousarray(
        np.tile(inp["gcn_b2"].reshape(E), 2).reshape(P, 1))
    rep["proj_b"] = np.ascontiguousarray(inp["proj_b"].reshape(E, 1))
    rep["attbr"] = np.ascontiguousarray(inp["att_b"].reshape(1, E))
    rep["projbr"] = np.ascontiguousarray(inp["proj_b"].reshape(1, E))
    rep["attv4"] = np.ascontiguousarray(
        np.tile(inp["att_vec"].reshape(1, E), (1, 4)))
    rep["mlp1_b"] = np.ascontiguousarray(inp["mlp1_b"].reshape(1, 16))
    rep["mlp2_b"] = np.ascontiguousarray(inp["mlp2_b"].reshape(1, 1))

    mnadj = inp["adj0"] + inp["adj1"]
    in_maps = []
    for r in range(C):
        rs = slice(r * NL, (r + 1) * NL)
        ms = slice(r * ML, (r + 1) * ML)
        d = dict(rep)
        d["feat1T"] = f8(_part3(inp["feat1"][ms].T))
        d["feat2T"] = f8(_part3(inp["feat2"][ms].T))
        d["nei0T"] = f8(_part3(inp["nei0"][:, ms].T))
        d["nei1T"] = f8(_part3(inp["nei1"][:, ms].T))
        d["recs"] = np.ascontiguousarray(np.stack(
            [1.0 / np.maximum(inp[f"nei{v}"][rs].sum(1), 1.0)
             for v in range(2)], axis=1).astype(np.float32))
        d["dmask"] = f8(np.ascontiguousarray(
            np.eye(N, dtype=np.float32)[rs]))
        d["f0m"] = f8(_part3(np.concatenate(
            [inp["feat0"][rs].T, inp["mask_feat"][rs].T], axis=1)))
        d["adj0T"] = f8(_part3(inp["adj0"][rs].T))
        d["adj1T"] = f8(_part3(inp["adj1"][rs].T))
        d["madj0T"] = f8(_part3(inp["madj0"][rs].T))
        d["madj1T"] = f8(_part3(inp["madj1"][rs].T))
        d["mnadjTf"] = f8(_part3(mnadj.T))
        in_maps.append(d)

    trace = bool(int(os.environ.get("KERNEL_TRACE", "0")))
    res = run_bass_kernel_spmd(
        nc, in_maps, core_ids=list(range(C)), trace=trace)
    if trace:
        _NC_CACHE["exec_time_ns"] = res.exec_time_ns
        _NC_CACHE["trace"] = res.instructions_and_trace
    total = 0.0
    for r in range(C):
        o = np.asarray(res.results[r]["out"], dtype=np.float64)
        total += float(np.sum(np.log(o[:, 0]) - np.log(2.0) - o[:, 1]))
    return np.float32(total / N)



# revision 42
# speedup vs baseline: 1.0565x; 1.0565x over previous
"""Distributed Trainium2 (Bass/Tile) kernel for the AdaMEOW GNN loss.

Sharding: target-node dim N row-sharded across 8 cores (128 rows each);
neighbor dim M sharded (512 each) for the neighbor-feature MLPs, combined
with one fp8 ReduceScatter (reciprocal neighbor counts are precomputed on
the host).  The h_tar/h_mask MLP runs FIRST so the y1-mean AllGather
doorbell rings as early as possible; the whole z_coarse chain (mean-adj
GCN, projection, l2-norm) is then computed FULL-N on every core inside
the ReduceScatter window - no z_coarse AllGather is needed, and the
pair-MLP B-matrix is ready before the RS lands.  Four collectives total:
AG(y1_mean), RS(neighbor agg partials), AG(fine y1), AG(fine y2), plus
the tiny attention-stats AG.  The [N,N,E] InfoNCE pair tensor is never
materialized: the pair-MLP is fused as
w[i,j] = sigmoid(sum_h tanh(A[i,h]+B[j,h])*m2[h]+b2), with sigmoid
computed via tanh to stay on one activation table; diag(logits) is
extracted with a host-provided 0/1 diagonal mask and a fused
tensor_tensor_reduce; all l2-norm rsqrts use a table-free quake-style
Newton iteration on the DVE.
"""

import os

import ml_dtypes
import numpy as np

import concourse.bass as bass
import concourse.mybir as mybir
import concourse.tile as tile
from concourse import bacc
from concourse.bass_utils import run_bass_kernel_spmd

FP = mybir.dt.float32
BF = mybir.dt.bfloat16
F8 = mybir.dt.float8e4
NPBF = ml_dtypes.bfloat16
NPF8 = ml_dtypes.float8_e4m3
AF = mybir.ActivationFunctionType
ALU = mybir.AluOpType
DR = mybir.MatmulPerfMode.DoubleRow

N, M, D0, D1, H, E = 1024, 4096, 1024, 512, 512, 64
C = 8            # cores
NL = N // C      # 128 local target nodes
ML = M // C      # 512 local neighbor nodes
P = 128
HK = H // P      # 4
D0K = D0 // P    # 8
MLK = ML // P    # 4
NB = N // P      # 8 node blocks
TAU = 0.5
RG = [list(range(C))]


def _build():
    nc = bacc.Bacc("TRN2", num_devices=C)

    def din(name, shape, dt=BF):
        return nc.declare_dram_parameter(name, list(shape), dt, isOutput=False)

    # per-core sharded inputs (host pre-arranged to final SBUF layouts)
    feat1T = din("feat1T", (P, MLK * ML), F8)     # [p, mlk, ML]
    feat2T = din("feat2T", (P, MLK * ML), F8)
    nei0T = din("nei0T", (P, MLK * N), F8)        # [p, mlk, N]
    nei1T = din("nei1T", (P, MLK * N), F8)
    recs = din("recs", (P, 2), FP)                # host 1/max(cnt,1) per view
    dmask = din("dmask", (P, N), F8)              # per-core diagonal mask
    f0m = din("f0m", (P, D0K * 2 * NL), F8)       # [p, k, tar|mask]
    adj0T = din("adj0T", (P, NB * NL), F8)
    adj1T = din("adj1T", (P, NB * NL), F8)
    madj0T = din("madj0T", (P, NB * NL), F8)
    madj1T = din("madj1T", (P, NB * NL), F8)
    mnadjTf = din("mnadjTf", (P, NB * N), F8)     # full (adj0+adj1).T
    # replicated weights
    fc0_w = din("fc0_w", (P, D0K * H), F8)
    fc1_w = din("fc1_w", (P, MLK * H), F8)
    fc2_w = din("fc2_w", (P, MLK * H), F8)
    agg0_w = din("agg0_w", (P, HK * H), F8)
    agg1_w = din("agg1_w", (P, HK * H), F8)
    gcn_w1 = din("gcn_w1", (P, HK * E), F8)
    gcn_w2 = din("gcn_w2", (E, E), BF)
    att_w = din("att_w", (E, E), BF)
    proj_w = din("proj_w", (E, E), BF)
    mlp1_w = din("mlp1_w", (E, 16), BF)
    sel8 = din("sel8", (16, P), BF)               # eye16 (x) ones(1,8)
    wm2g = din("wm2g", (P, 8 * 64), BF)           # padded m2-kron blocks
    eye128 = din("eye128", (P, P), BF)
    # small aux tensors
    fc0_b = din("fc0_b", (P, HK), FP)             # [p, hc] feature-partition
    fc1_b = din("fc1_b", (1, H), BF)              # row (for psum bias init)
    fc2_b = din("fc2_b", (1, H), BF)
    gcn_b1 = din("gcn_b1", (P, 1), FP)            # tiled x2 -> [128,1]
    gcn_b2 = din("gcn_b2", (P, 1), FP)
    attbr = din("attbr", (1, E), FP)              # att_b as a row
    projbr = din("projbr", (1, E), FP)            # proj_b as a row
    attv4 = din("attv4", (1, 4 * E), FP)          # att_vec tiled x4
    proj_b = din("proj_b", (E, 1), FP)
    mlp1_b = din("mlp1_b", (1, 16), FP)
    mlp2_b = din("mlp2_b", (1, 1), FP)

    out_ext = nc.declare_dram_parameter("out", [NL, 2], FP, isOutput=True)

    # collective bounce buffers
    agm_in = nc.dram_tensor("agm_in", [NL, E], F8)
    agm_out = nc.dram_tensor("agm_out", [N, E], F8, addr_space="Shared")
    rs_in = nc.dram_tensor("rs_in", [NB, P, 2 * HK, P], F8)
    rs_out = nc.dram_tensor("rs_out", [P, 2 * HK, P], F8)
    ag1_in = nc.dram_tensor("ag1_in", [NL, 4 * E], F8)
    ag1_out = nc.dram_tensor("ag1_out", [N, 4 * E], F8, addr_space="Shared")
    ag2_in = nc.dram_tensor("ag2_in", [NL, 4 * E], F8)
    ag2_out = nc.dram_tensor("ag2_out", [N, 4 * E], F8, addr_space="Shared")
    ag3a_in = nc.dram_tensor("ag3a_in", [1, P], BF)
    dramA = nc.dram_tensor("dramA", [NL, 16], FP)
    dramRN = nc.dram_tensor("dramRN", [1, N], BF)
    ag3a_out = nc.dram_tensor("ag3a_out", [C, P], BF, addr_space="Shared")

    with tile.TileContext(nc) as tc:
        with (
            tc.tile_pool(name="pers", bufs=1) as pers,
            tc.tile_pool(name="wkE", bufs=4) as wkE,
            tc.tile_pool(name="wkT", bufs=4) as wkT,
            tc.tile_pool(name="wkS", bufs=2) as wkS,
        ):
            def mk(pool, shape, name, dt=FP):
                return pool.tile(list(shape), dt, tag=name, name=name)

            def ld(pool, dram, shape, name, eng=None):
                t = mk(pool, shape, name, dt=dram.dtype)
                src = dram[:]
                if list(t.shape) != list(dram.shape):
                    src = src.rearrange("p (a b) -> p a b", a=t.shape[1])
                (eng or nc.sync).dma_start(t[:], src)
                return t

            def elu(ps_ap, ebias=0.0):
                """elu(x) = relu(x) + min(exp(x), 1) - 1; 2 ACT + 1 DVE."""
                sh = [ps_ap.shape[0], ps_ap.free_size()]
                e = wkE.tile(sh, BF, tag="elu_e", name="elu_e")
                r = wkE.tile(sh, BF, tag="elu_r", name="elu_r")
                nc.scalar.activation(e[:], ps_ap, AF.Exp, bias=ebias)
                nc.scalar.activation(r[:], ps_ap, AF.Relu, bias=ebias)
                q = wkE.tile(sh, BF, tag="elu_q", name="elu_q")
                nc.vector.tensor_scalar(
                    out=q[:], in0=e[:], scalar1=1.0, scalar2=-1.0,
                    op0=ALU.min, op1=ALU.add)
                return q, r

            def rsqrt_tile(ps_ap, p, f, tag, out_dt=BF):
                """Table-free rsqrt of a [p, f] psum/sbuf tile (quake
                seed + one Newton step on DVE)."""
                x = wkS.tile([p, f], FP, tag="rsq_x", name=tag + "x")
                nc.vector.tensor_scalar_max(x[:], ps_ap, 1e-24)
                sh = wkS.tile([p, f], FP, tag="rsq_s", name=tag + "s")
                nc.vector.tensor_scalar(
                    out=sh[:].bitcast(mybir.dt.uint32),
                    in0=x[:].bitcast(mybir.dt.uint32),
                    scalar1=1, scalar2=0,
                    op0=ALU.logical_shift_right, op1=ALU.bitwise_or)
                y = wkS.tile([p, f], FP, tag="rsq_y", name=tag + "y")
                nc.vector.tensor_tensor(
                    out=y[:].bitcast(mybir.dt.uint32),
                    in0=magic_sb[0:p, 0:f].bitcast(mybir.dt.uint32),
                    in1=sh[:].bitcast(mybir.dt.uint32),
                    op=ALU.subtract)
                # Newton step(s): y *= 1.5 - 0.5 x y^2
                for it in range(1):
                    t = wkS.tile([p, f], FP, tag="rsq_t", name=tag + "t")
                    nc.vector.tensor_tensor(out=t[:], in0=y[:], in1=y[:],
                                            op=ALU.mult)
                    nc.vector.tensor_tensor(out=t[:], in0=t[:], in1=x[:],
                                            op=ALU.mult)
                    nc.vector.tensor_scalar(
                        out=t[:], in0=t[:], scalar1=-0.5, scalar2=1.5,
                        op0=ALU.mult, op1=ALU.add)
                    nc.vector.tensor_tensor(out=y[:], in0=y[:], in1=t[:],
                                            op=ALU.mult)
                if out_dt == FP:
                    return y
                yb = wkS.tile([p, f], out_dt, tag="rsq_b", name=tag + "b")
                nc.vector.tensor_copy(yb[:], y[:])
                return yb

            def rsqrt_row(ps_ap, nl, tag):
                return rsqrt_tile(ps_ap, 1, nl, tag)

            # ---------------- persistent constants --------------------
            ones_row = mk(pers, (1, 512), "ones_row", BF)
            nc.vector.memset(ones_row[:], 1.0)
            ones_col = mk(pers, (P, 1), "ones_col", BF)
            nc.vector.memset(ones_col[:], 1.0)
            magic_sb = mk(pers, (P, 512), "magic", FP)
            nc.vector.memset(magic_sb[:].bitcast(mybir.dt.uint32),
                             0x5f3759df)

            def ld2(pool, dram, shape, name, eng0, eng1):
                """Split a [P, a, b] load into two halves on two engine
                queues so the DMA rings drain it in parallel."""
                t = mk(pool, shape, name, dt=dram.dtype)
                src = dram[:].rearrange("p (a b) -> p a b", a=shape[1])
                h = shape[1] // 2
                eng0.dma_start(t[:, 0:h, :], src[:, 0:h, :])
                eng1.dma_start(t[:, h:, :], src[:, h:, :])
                return t

            # ================= stage 1: fp8 MLPs + aggregation ========
            f0m_sb = ld(pers, f0m, (P, D0K, 2 * NL), "f0m", nc.sync)
            fc0w_sb = ld(pers, fc0_w, (P, D0K, H), "fc0w", nc.gpsimd)
            fc0b_sb = ld(pers, fc0_b, (P, HK), "fc0b", nc.scalar)
            gcnw1_sb = ld(pers, gcn_w1, (P, HK, E), "gcnw1", nc.scalar)
            feat1T_sb = ld(pers, feat1T, (P, MLK, ML), "feat1T", nc.gpsimd)
            fc1w_sb = ld(pers, fc1_w, (P, MLK, H), "fc1w", nc.sync)
            feat2T_sb = ld(pers, feat2T, (P, MLK, ML), "feat2T", nc.gpsimd)
            fc2w_sb = ld(pers, fc2_w, (P, MLK, H), "fc2w", nc.sync)
            nei0T_sb = ld(pers, nei0T, (P, MLK, N), "nei0T", nc.gpsimd)
            nei1T_sb = ld(pers, nei1T, (P, MLK, N), "nei1T", nc.sync)
            fc1b_sb = ld(pers, fc1_b, (1, H), "fc1b", nc.scalar)
            fc2b_sb = ld(pers, fc2_b, (1, H), "fc2b", nc.scalar)

            hnei_sb = [mk(pers, (P, MLK, H), "hnei0", F8),
                       mk(pers, (P, MLK, H), "hnei1", F8)]

            with tc.tile_pool(name="psA", bufs=3, space="PSUM") as psA:
                # ---- h[tar|mask]T + y1_mean FIRST: rings the y1-mean
                # AllGather doorbell as early as possible so the AG mesh
                # completes before rs_in is ready (un-gates the RS) ----
                hthm_sb = mk(pers, (P, HK, 4 * NL), "hthm", BF)
                for hc in range(HK):
                    ps = psA.tile([P, 2 * NL], FP, tag="psA", name="ps_ht")
                    for kp in range(D0K // 2):
                        nc.tensor.matmul(
                            ps[:],
                            fc0w_sb[:, 2 * kp:2 * kp + 2,
                                    hc * P:(hc + 1) * P],
                            f0m_sb[:, 2 * kp:2 * kp + 2, :],
                            start=(kp == 0), stop=(kp == D0K // 2 - 1),
                            perf_mode=DR)
                    q, r = elu(ps[:], ebias=fc0b_sb[:, hc:hc + 1])
                    nc.vector.tensor_tensor(
                        out=hthm_sb[:, hc, 0:2 * NL], in0=q[:], in1=r[:],
                        op=ALU.add)
                htar8 = mk(pers, (P, HK, NL), "htar8", F8)
                nc.vector.tensor_copy(htar8[:], hthm_sb[:, :, 0:NL])
                psm = psA.tile([P, E], FP, tag="psA", name="ps_y1m")
                for kp in range(HK // 2):
                    nc.tensor.matmul(
                        psm[:], htar8[:, 2 * kp:2 * kp + 2, :],
                        gcnw1_sb[:, 2 * kp:2 * kp + 2, :],
                        start=(kp == 0), stop=(kp == HK // 2 - 1),
                        perf_mode=DR)
                stm = wkS.tile([NL, E], F8, tag="stm", name="stm")
                nc.vector.tensor_scalar_mul(stm[:], psm[:], 0.5)
                nc.sync.dma_start(agm_in[:], stm[:])

                # ---- h_nei shards: elu(featX @ fcX_w + b) in fp8 -----
                for v, (fT, fw, fb) in enumerate(
                    [(feat1T_sb, fc1w_sb, fc1b_sb),
                     (feat2T_sb, fc2w_sb, fc2b_sb)]
                ):
                    for mc in range(MLK):
                        ps = psA.tile([P, H], FP, tag="psA", name="ps_hnei")
                        nc.tensor.matmul(ps[:], ones_row[:, 0:P], fb[:],
                                         start=True, stop=False)
                        for kp in range(MLK // 2):
                            nc.tensor.matmul(
                                ps[:],
                                fT[:, 2 * kp:2 * kp + 2, mc * P:(mc + 1) * P],
                                fw[:, 2 * kp:2 * kp + 2, :],
                                start=False, stop=(kp == MLK // 2 - 1),
                                perf_mode=DR)
                        q, r = elu(ps[:])
                        nc.vector.tensor_tensor(
                            out=hnei_sb[v][:, mc, :], in0=q[:], in1=r[:],
                            op=ALU.add)

                for hc in range(HK):
                    nc.vector.tensor_copy(hthm_sb[:, hc, 2 * NL:4 * NL],
                                          hthm_sb[:, hc, 0:2 * NL])

                # ---- partial aggregation (feature-major, fp8) --------
                wq = [nc.sync, nc.scalar]
                for v, neiT in enumerate([nei0T_sb, nei1T_sb]):
                    for hc in range(HK):
                        for jh in range(2):
                            ps = psA.tile([P, 512], FP, tag="psA",
                                          name="ps_pr")
                            for kp in range(MLK // 2):
                                nc.tensor.matmul(
                                    ps[:],
                                    hnei_sb[v][:, 2 * kp:2 * kp + 2,
                                               hc * P:(hc + 1) * P],
                                    neiT[:, 2 * kp:2 * kp + 2,
                                         jh * 512:(jh + 1) * 512],
                                    start=(kp == 0),
                                    stop=(kp == MLK // 2 - 1),
                                    perf_mode=DR)
                            prs = wkE.tile([P, 512], F8, tag="prs",
                                           name="prs")
                            if (hc + jh) % 2 == 0:
                                nc.vector.tensor_copy(prs[:], ps[:])
                            else:
                                nc.scalar.activation(prs[:], ps[:],
                                                     AF.Copy)
                            wq[(hc + jh) % 2].dma_start(
                                rs_in[jh * 4:(jh + 1) * 4, :,
                                      v * HK + hc:v * HK + hc + 1,
                                      :].rearrange(
                                          "b p one n -> p b (one n)"),
                                prs[:].rearrange("p (b n) -> p b n", b=4))

            # cc stream order: AG(y1_mean) -> RS -> AG(zc) -> AG1 ...
            nc.gpsimd.collective_compute(
                "AllGather", ALU.bypass, replica_groups=RG,
                ins=[agm_in[:].opt()], outs=[agm_out[:].opt()])
            nc.gpsimd.collective_compute(
                "ReduceScatter", ALU.add, replica_groups=RG,
                ins=[rs_in[:].opt()], outs=[rs_out[:].opt()])

            # ============ phase 1 (overlaps the ReduceScatter) ========
            with tc.tile_pool(name="psB", bufs=3, space="PSUM") as psB, \
                 tc.tile_pool(name="psS", bufs=4, space="PSUM") as psS:
                adj0T_sb = ld(pers, adj0T, (P, NB, NL), "adj0T", nc.sync)
                adj1T_sb = ld(pers, adj1T, (P, NB, NL), "adj1T", nc.scalar)
                madj0T_sb = ld(pers, madj0T, (P, NB, NL), "madj0T", nc.sync)
                madj1T_sb = ld(pers, madj1T, (P, NB, NL), "madj1T",
                               nc.scalar)
                mnadjTf_sb = ld(pers, mnadjTf, (P, NB, N), "mnadjTf",
                                nc.sync)
                agg0w_sb = ld(pers, agg0_w, (P, HK, H), "agg0w", nc.sync)
                agg1w_sb = ld(pers, agg1_w, (P, HK, H), "agg1w", nc.scalar)
                recs_sb = ld(pers, recs, (P, 2), "recs", nc.sync)
                dmask_sb = ld(pers, dmask, (P, N), "dmask", nc.scalar)
                gcnw2_sb = ld(pers, gcn_w2, (E, E), "gcnw2", nc.sync)
                attw_sb = ld(pers, att_w, (E, E), "attw", nc.sync)
                projw_sb = ld(pers, proj_w, (E, E), "projw", nc.sync)
                mlp1w_sb = ld(pers, mlp1_w, (E, 16), "mlp1w", nc.sync)
                sel8_sb = ld(pers, sel8, (16, P), "sel8", nc.scalar)
                wm2g_sb = ld(pers, wm2g, (P, 8, 64), "wm2g", nc.scalar)
                eye_sb = ld(pers, eye128, (P, P), "eye", nc.scalar)
                gcnb1_sb = ld(pers, gcn_b1, (P, 1), "gcnb1", nc.sync)
                gcnb2_sb = ld(pers, gcn_b2, (P, 1), "gcnb2", nc.sync)
                projb_sb = ld(pers, proj_b, (E, 1), "projb", nc.sync)
                attb_bcT = mk(pers, (P, E), "attb_bcT")
                nc.sync.dma_start(attb_bcT[:], attbr[:].to_broadcast((P, E)))
                projb_bcT = mk(pers, (P, E), "projb_bcT")
                nc.sync.dma_start(projb_bcT[:],
                                  projbr[:].to_broadcast((P, E)))
                attv4_sb = ld(pers, attv4, (1, 4 * E), "attv4", nc.scalar)
                b1bc16 = mk(pers, (P, 16), "b1bc16")
                nc.sync.dma_start(b1bc16[:], mlp1_b[:].to_broadcast((P, 16)))

                b2h = mk(pers, (P, 1), "b2h")
                nc.sync.dma_start(b2h[:], mlp2_b[:].to_broadcast((P, 1)))
                nc.vector.tensor_scalar_mul(b2h[:], b2h[:], 0.5)

                # reciprocal counts precomputed on host (recs input)
                rec4 = []
                for v in range(2):
                    rcb = wkS.tile([P, 1], BF, tag="rcb", name="rcb")
                    nc.vector.tensor_copy(rcb[:], recs_sb[:, v:v + 1])
                    pst = psS.tile([1, P], FP, tag="psS", name="ps_rT")
                    nc.tensor.matmul(pst[:], rcb[:], eye_sb[:])
                    rrow4 = wkS.tile([1, 4, P], BF, tag="rrow4",
                                     name="rrow4")
                    for t4 in range(4):
                        nc.vector.tensor_copy(rrow4[:, t4, :], pst[:])
                    psb = psB.tile([P, 4 * P], FP, tag="psB", name="ps_rbc")
                    nc.tensor.matmul(psb[:], ones_row[:, 0:P],
                                     rrow4[:].rearrange("o a b -> o (a b)"))
                    rb = mk(pers, (P, 4 * P), f"rec4_{v}", BF)
                    nc.vector.tensor_copy(rb[:], psb[:])
                    rec4.append(rb)

                # ---- z_coarse chain: full-N mean conv (in RS window) -
                y1m_sb = mk(pers, (P, NB, E), "y1mall", F8)
                nc.sync.dma_start(
                    y1m_sb[:], agm_out[:].rearrange("(b p) e -> p b e", p=P))
                hmT_sb = mk(pers, (E, N), "hmT", F8)
                for jh in range(2):
                    ps = psB.tile([P, 512], FP, tag="psB", name="ps_hm")
                    for bp in range(NB // 2):
                        nc.tensor.matmul(
                            ps[0:E, :], y1m_sb[:, 2 * bp:2 * bp + 2, :],
                            mnadjTf_sb[:, 2 * bp:2 * bp + 2,
                                       jh * 512:(jh + 1) * 512],
                            start=(bp == 0), stop=(bp == NB // 2 - 1),
                            perf_mode=DR)
                    nc.vector.tensor_scalar(
                        out=hmT_sb[:, jh * 512:(jh + 1) * 512],
                        in0=ps[0:E, :], scalar1=gcnb1_sb[0:E, :],
                        scalar2=0.0, op0=ALU.add, op1=ALU.max)
                # y2_mean (x0.5 for the mean-adj sum) [p, NB, E] fp8
                y2m_sb = mk(pers, (P, NB, E), "y2m", F8)
                for b in range(NB):
                    ps = psS.tile([P, E], FP, tag="psS", name="ps_y2m")
                    nc.tensor.matmul(ps[:], hmT_sb[:, b * P:(b + 1) * P],
                                     gcnw2_sb[:])
                    nc.vector.tensor_scalar_mul(y2m_sb[:, b, :], ps[:], 0.5)
                # ---- z_coarse FULL-N (replicated on every core; no
                # AllGather needed): conv with full mean-adj, then
                # proj+tanh+colnorm per 512-col half -> zcall [E, N] ---
                zT_sb = mk(pers, (E, 4, NL), "zT", BF)
                zcall_sb = mk(pers, (E, N), "zcall", BF)
                BT_sb = mk(pers, (16, N), "BT", BF)
                tfcall = mk(pers, (E, N), "tfcall", BF)
                for jh in range(2):
                    pszm = psB.tile([P, 512], FP, tag="psB", name="ps_zm")
                    for bp in range(NB // 2):
                        nc.tensor.matmul(
                            pszm[0:E, :], y2m_sb[:, 2 * bp:2 * bp + 2, :],
                            mnadjTf_sb[:, 2 * bp:2 * bp + 2,
                                       jh * 512:(jh + 1) * 512],
                            start=(bp == 0), stop=(bp == NB // 2 - 1),
                            perf_mode=DR)
                    zcf = wkS.tile([E, 512], BF, tag="zcf", name="zcf")
                    nc.vector.tensor_scalar_add(zcf[:], pszm[0:E, :],
                                                gcnb2_sb[0:E, :])
                    psz = psB.tile([P, 512], FP, tag="psB", name="ps_pzc")
                    nc.tensor.matmul(psz[0:E, :], projw_sb[:], zcf[:])
                    nc.scalar.activation(
                        tfcall[:, jh * 512:(jh + 1) * 512], psz[0:E, :],
                        AF.Tanh, bias=projb_sb[:])
                # node-major norms: transpose 128-col blocks, square on
                # ACT, row-reduce -> [P, NB]; rsqrt with free-dim 8 is
                # ~100x cheaper than the [1, N] row variant on the DVE
                nzc = wkS.tile([P, NB], FP, tag="nzc", name="nzc")
                for q in range(4):
                    tq = psS.tile([P, 2, E], FP, tag="psS", name="ps_tq")
                    for s in range(2):
                        b = 2 * q + s
                        nc.tensor.matmul(tq[:, s, :],
                                         tfcall[:, b * P:(b + 1) * P],
                                         eye_sb[0:E, 0:E])
                    sqb = wkE.tile([P, 2, E], BF, tag="sqv", name="sqb")
                    nc.scalar.activation(
                        sqb[:].rearrange("p a b -> p (a b)"),
                        tq[:].rearrange("p a b -> p (a b)"), AF.Square)
                    nc.vector.reduce_sum(nzc[:, 2 * q:2 * q + 2], sqb[:],
                                         axis=mybir.AxisListType.X)
                rz8 = rsqrt_tile(nzc[:], P, NB, "rz8")
                nc.sync.dma_start(
                    dramRN[:].rearrange("o (b n) -> n (o b)", n=P), rz8[:])
                rnrow = wkS.tile([1, N], BF, tag="rnrow", name="rnrow")
                nc.sync.dma_start(rnrow[:], dramRN[:])
                for jh in range(2):
                    hf = slice(jh * 512, (jh + 1) * 512)
                    psbz = psB.tile([P, 512], FP, tag="psB", name="ps_nbz")
                    nc.tensor.matmul(psbz[0:E, :], ones_row[:, 0:E],
                                     rnrow[:, hf])
                    nc.vector.tensor_mul(zcall_sb[:, hf], tfcall[:, hf],
                                         psbz[0:E, :])
                    pbt = psB.tile([P, 512], FP, tag="psB", name="ps_BT")
                    nc.tensor.matmul(pbt[0:16, :], mlp1w_sb[:],
                                     zcall_sb[:, hf])
                    nc.vector.tensor_copy(BT_sb[:, hf], pbt[0:16, :])

                # ================= post-RS: views + fine GCN ==========
                aggT_sb = mk(pers, (P, 2 * HK, NL), "aggT", F8)
                nc.sync.dma_start(aggT_sb[:, 0:HK, :], rs_out[:, 0:HK, :])
                nc.scalar.dma_start(aggT_sb[:, HK:, :], rs_out[:, HK:, :])
                aggS_sb = mk(pers, (P, 2 * HK, 2 * NL), "aggS", F8)
                for v in range(2):
                    for half in range(2):
                        nc.vector.tensor_tensor(
                            out=aggS_sb[:, v * HK:(v + 1) * HK,
                                        half * NL:(half + 1) * NL],
                            in0=aggT_sb[:, v * HK:(v + 1) * HK, :],
                            in1=rec4[v][:].rearrange(
                                "p (a b) -> p a b", a=HK),
                            op=ALU.mult)

                # both views + masks in one [P, 512] pass per h-chunk:
                # cols [v0tar | v0mask | v1tar | v1mask]
                xs4 = mk(pers, (P, HK, 4 * NL), "xs4", F8)
                for hc in range(HK):
                    ps = psB.tile([P, 4 * NL], FP, tag="psB", name="ps_x2")
                    for v, aggw in enumerate([agg0w_sb, agg1w_sb]):
                        half = ps[:, v * 2 * NL:(v + 1) * 2 * NL]
                        nc.tensor.matmul(
                            half, eye_sb[:],
                            hthm_sb[:, hc, v * 2 * NL:(v + 1) * 2 * NL],
                            start=True, stop=False)
                        for kp in range(HK // 2):
                            nc.tensor.matmul(
                                half,
                                aggw[:, 2 * kp:2 * kp + 2,
                                     hc * P:(hc + 1) * P],
                                aggS_sb[:, v * HK + 2 * kp:
                                        v * HK + 2 * kp + 2, :],
                                start=False, stop=(kp == HK // 2 - 1),
                                perf_mode=DR)
                    q, r = elu(ps[:])
                    nc.vector.tensor_tensor(
                        out=xs4[:, hc, :], in0=q[:], in1=r[:], op=ALU.add)

                # GCN layer-1 linear; st4a cols [v0, v1, m0, m1]
                st4a = mk(pers, (NL, 4, E), "st4a", F8)
                for c0, slot in [(0, 0), (2 * NL, 1), (NL, 2), (3 * NL, 3)]:
                    ps = psS.tile([P, E], FP, tag="psS", name="ps_y1")
                    for kp in range(HK // 2):
                        nc.tensor.matmul(
                            ps[:], xs4[:, 2 * kp:2 * kp + 2, c0:c0 + NL],
                            gcnw1_sb[:, 2 * kp:2 * kp + 2, :],
                            start=(kp == 0), stop=(kp == HK // 2 - 1),
                            perf_mode=DR)
                    nc.vector.tensor_copy(st4a[:, slot, :], ps[:])
                nc.sync.dma_start(
                    ag1_in[:].rearrange("n (g e) -> n g e", g=4), st4a[:])
                nc.gpsimd.collective_compute(
                    "AllGather", ALU.bypass, replica_groups=RG,
                    ins=[ag1_in[:].opt()], outs=[ag1_out[:].opt()])

                def conv_fine(y_sb, badd, relu, outs):
                    """4 fine graph convs; y_sb [P, NB, 4E] fp8 cols
                    [v0, v1, m0, m1]; outs: list of 4 (dst_ap)."""
                    pp = [psS.tile([E, NL], FP, tag="psS", name=f"pc{g}")
                          for g in range(4)]
                    adjs = [adj0T_sb, adj1T_sb, madj0T_sb, madj1T_sb]
                    for bp in range(NB // 2):
                        for g in range(4):
                            nc.tensor.matmul(
                                pp[g][:],
                                y_sb[:, 2 * bp:2 * bp + 2,
                                     g * E:(g + 1) * E],
                                adjs[g][:, 2 * bp:2 * bp + 2, :],
                                start=(bp == 0), stop=(bp == NB // 2 - 1),
                                perf_mode=DR)
                    op1 = ALU.max if relu else ALU.bypass
                    for g in range(4):
                        nc.vector.tensor_scalar(
                            out=outs[g], in0=pp[g][:],
                            scalar1=badd[0:E, :], scalar2=0.0,
                            op0=ALU.add, op1=op1)

                y1_sb = mk(pers, (P, NB, 4 * E), "y1", F8)
                y1src = ag1_out[:].rearrange("(b p) f -> p b f", p=P)
                for qi, qe in enumerate([nc.sync, nc.scalar,
                                         nc.sync, nc.scalar]):
                    qe.dma_start(y1_sb[:, 2 * qi:2 * qi + 2, :],
                                 y1src[:, 2 * qi:2 * qi + 2, :])
                h4_sb = mk(pers, (E, 4, NL), "h4", BF)
                conv_fine(y1_sb, gcnb1_sb, True,
                          [h4_sb[:, g, :] for g in range(4)])
                st4b = mk(pers, (NL, 4, E), "st4b", F8)
                for g in range(4):
                    ps = psS.tile([P, E], FP, tag="psS", name="ps_y2")
                    nc.tensor.matmul(ps[:], h4_sb[:, g, :], gcnw2_sb[:])
                    nc.vector.tensor_copy(st4b[:, g, :], ps[:])
                nc.sync.dma_start(
                    ag2_in[:].rearrange("n (g e) -> n g e", g=4), st4b[:])
                nc.gpsimd.collective_compute(
                    "AllGather", ALU.bypass, replica_groups=RG,
                    ins=[ag2_in[:].opt()], outs=[ag2_out[:].opt()])

                y2_sb = mk(pers, (P, NB, 4 * E), "y2", F8)
                y2src = ag2_out[:].rearrange("(b p) f -> p b f", p=P)
                for qi, qe in enumerate([nc.sync, nc.scalar,
                                         nc.sync, nc.scalar]):
                    qe.dma_start(y2_sb[:, 2 * qi:2 * qi + 2, :],
                                 y2src[:, 2 * qi:2 * qi + 2, :])
                # conv2 -> zT slots [v0, v1, m0, m1] -> [v0, m0, v1, m1]
                conv_fine(y2_sb, gcnb2_sb, False,
                          [zT_sb[:, 0, :], zT_sb[:, 2, :],
                           zT_sb[:, 1, :], zT_sb[:, 3, :]])

                # ---- attention, node-major: every per-node scalar
                # (norm, beta-weight) lives on partitions so the rsqrt
                # and scale ops run full-lane instead of on one row ----
                attT = psS.tile([P, 4, E], FP, tag="psS", name="ps_attT")
                z4T = psS.tile([P, 4, E], FP, tag="psS", name="ps_z4T")
                pj4T = psS.tile([P, 4, E], FP, tag="psS", name="ps_pj4T")
                for v in range(4):
                    zv = zT_sb[:, v, :]
                    nc.tensor.matmul(attT[:, v, :], zv, attw_sb[:])
                    nc.tensor.matmul(z4T[:, v, :], zv,
                                     eye_sb[0:E, 0:E])
                    nc.tensor.matmul(pj4T[:, v, :], zv, projw_sb[:])
                norm4 = wkS.tile([P, 4], FP, tag="norm4", name="norm4")
                sq4T = wkS.tile([P, 4, E], BF, tag="sq4T", name="sq4T")
                nc.scalar.activation(
                    sq4T[:].rearrange("p a b -> p (a b)"),
                    z4T[:].rearrange("p a b -> p (a b)"), AF.Square)
                nc.vector.reduce_sum(norm4[:], sq4T[:],
                                     axis=mybir.AxisListType.X)
                rn4T = rsqrt_tile(norm4[:], P, 4, "rn4", out_dt=FP)
                taT = wkS.tile([P, 4, E], BF, tag="taT", name="taT")
                for v in range(4):
                    nc.vector.scalar_tensor_tensor(
                        out=taT[:, v, :], in0=attT[:, v, :],
                        scalar=rn4T[:, v:v + 1], in1=attb_bcT[:],
                        op0=ALU.mult, op1=ALU.add)
                taTt = wkS.tile([P, 4, E], BF, tag="taTt", name="taTt")
                nc.scalar.activation(
                    taTt[:].rearrange("p a b -> p (a b)"),
                    taT[:].rearrange("p a b -> p (a b)"), AF.Tanh)
                psE4 = psS.tile([1, 4, E], FP, tag="psSt",
                                name="ps_e4", bufs=1)
                for v in range(4):
                    nc.tensor.matmul(psE4[:, v, :], ones_col[:],
                                     taTt[:, v, :])
                se4 = wkS.tile([1, 4, E], FP, tag="se4", name="se4")
                nc.vector.tensor_mul(
                    se4[:].rearrange("o a b -> o (a b)"),
                    psE4[:].rearrange("o a b -> o (a b)"), attv4_sb[:])
                er4 = wkS.tile([1, 4], FP, tag="er4", name="er4")
                nc.vector.reduce_sum(er4[:], se4[:],
                                     axis=mybir.AxisListType.X)
                e_row = wkS.tile([1, P], BF, tag="e_row", name="e_row")
                nc.vector.memset(e_row[:], 0.0)
                nc.vector.tensor_scalar_mul(e_row[:, 0:4], er4[:], 1.0 / N)
                nc.sync.dma_start(ag3a_in[:], e_row[:])
                nc.gpsimd.collective_compute(
                    "AllGather", ALU.bypass, replica_groups=RG,
                    ins=[ag3a_in[:].opt()], outs=[ag3a_out[:].opt()])

                # ---- softmax over views; z_fine (node-major); A ------
                e8_sb = wkS.tile([C, 4], BF, tag="e8", name="e8")
                nc.sync.dma_start(e8_sb[:], ag3a_out[:, 0:4])
                pse2 = psS.tile([1, 4], FP, tag="psSt", name="ps_e2",
                                bufs=1)
                nc.tensor.matmul(pse2[:], ones_col[0:C, :], e8_sb[:])
                ee = wkS.tile([1, 4], FP, tag="ee", name="ee")
                nc.scalar.activation(ee[:], pse2[:], AF.Exp)
                se = wkS.tile([1, 1], FP, tag="se", name="se")
                nc.vector.reduce_sum(se[:], ee[:], axis=mybir.AxisListType.X)
                nc.vector.reciprocal(se[:], se[:])
                beta_row = wkS.tile([1, 4], BF, tag="beta", name="beta")
                nc.vector.tensor_scalar_mul(beta_row[:], ee[:], se[:])
                psbb = psS.tile([P, 4], FP, tag="psSt",
                                name="ps_beta", bufs=1)
                nc.tensor.matmul(psbb[:], ones_row[:, 0:P], beta_row[:])
                rnb4 = wkS.tile([P, 4], FP, tag="rnb4", name="rnb4")
                nc.vector.tensor_mul(rnb4[:], rn4T[:], psbb[:])
                zfpT = wkS.tile([P, E], FP, tag="zfpT", name="zfpT")
                nc.vector.tensor_scalar(
                    out=zfpT[:], in0=pj4T[:, 0, :], scalar1=rnb4[:, 0:1],
                    scalar2=0.0, op0=ALU.mult, op1=ALU.add)
                for v in range(1, 4):
                    nc.vector.scalar_tensor_tensor(
                        out=zfpT[:], in0=pj4T[:, v, :],
                        scalar=rnb4[:, v:v + 1], in1=zfpT[:],
                        op0=ALU.mult, op1=ALU.add)
                tf2T = wkS.tile([P, E], BF, tag="tf2T", name="tf2T")
                nc.vector.tensor_add(zfpT[:], zfpT[:], projb_bcT[:])
                nc.scalar.activation(tf2T[:], zfpT[:], AF.Tanh)
                # 1/||zf|| per anchor, already on partitions
                nrm1 = wkS.tile([P, 1], FP, tag="nrm1", name="nrm1")
                sqf = wkE.tile([P, E], BF, tag="sqv", name="sqf")
                nc.vector.scalar_tensor_tensor(
                    out=sqf[:], in0=tf2T[:], scalar=1.0, in1=tf2T[:],
                    op0=ALU.bypass, op1=ALU.mult, accum_out=nrm1[:])
                recn = rsqrt_tile(nrm1[:], P, 1, "rzf", out_dt=FP)
                rec2 = mk(pers, (P, 1), "rec2")
                nc.vector.tensor_scalar_mul(rec2[:], recn[:], 1.0 / TAU)
                # transpose tf2T back to [E, NL] for the logits matmuls
                tf2ps = psS.tile([E, NL], FP, tag="psSt",
                                 name="ps_tf2", bufs=1)
                nc.tensor.matmul(tf2ps[:], tf2T[:], eye_sb[:])
                tf2 = mk(pers, (E, NL), "tf2", BF)
                nc.vector.tensor_copy(tf2[:], tf2ps[:])
                psa2 = psS.tile([NL, 16], FP, tag="psSt",
                                name="ps_A", bufs=1)
                nc.tensor.matmul(psa2[:], tf2[:], mlp1w_sb[:])
                A_sb = mk(pers, (NL, 16), "A")
                nc.vector.scalar_tensor_tensor(
                    out=A_sb[:], in0=psa2[:], scalar=recn[:, 0:1],
                    in1=b1bc16[:], op0=ALU.mult, op1=ALU.add)
                # rearrange A[i,h] -> A_all[(h*8+k), ic] (i = ic*8+k) via
                # a DRAM round-trip so the pair-MLP h-sum becomes a PE
                # contraction instead of 16 DVE multiply-adds
                nc.sync.dma_start(dramA[:], A_sb[:])
                A_all = mk(pers, (P, 16), "A_all")
                nc.sync.dma_start(
                    A_all[:],
                    dramA[:].rearrange("(ic k) h -> (k h) ic", k=8))

            # ================= InfoNCE tail (2-bank psum) =============
            with tc.tile_pool(name="psT", bufs=3, space="PSUM") as psT:
                # dots = exp((zfn.zc) / tau)  (exp/tanh table)
                psl = psT.tile([P, N], FP, tag="psT", name="ps_log")
                for jh in range(2):
                    nc.tensor.matmul(
                        psl[:, jh * 512:(jh + 1) * 512], tf2[:],
                        zcall_sb[:, jh * 512:(jh + 1) * 512])
                dots_sb = mk(pers, (P, N), "dots", BF)
                nc.scalar.activation(dots_sb[:], psl[:], AF.Exp,
                                     scale=rec2[:, 0:1])
                s0_sb = wkS.tile([P, 1], FP, tag="s0", name="s0")
                nc.vector.reduce_sum(s0_sb[:], dots_sb[:],
                                     axis=mybir.AxisListType.X)
                # diag(logits): mask out the per-core diagonal of the
                # raw logit block and row-reduce; scaled by rec2 at the
                # end (replaces the separate local-zc dot product)
                scrd = wkT.tile([P, N], BF, tag="th", name="scrd")
                dgr = wkS.tile([P, 1], FP, tag="dgr", name="dgr")
                nc.vector.scalar_tensor_tensor(
                    out=scrd[:], in0=psl[:], scalar=1.0, in1=dmask_sb[:],
                    op0=ALU.bypass, op1=ALU.mult, accum_out=dgr[:])

                # acc[i,j] = sum_h tanh(A[i,h] + B[j,h]) * m2[h]:
                # partitions hold (h*8+k) for an 8-anchor chunk; ONE
                # B-broadcast psum serves all 16 chunks, each chunk is
                # one tanh (bias = A_all column) + a PE contraction over
                # the 16 h-lanes via the block-diagonal wm2, and the
                # [8, N] results DMA back to anchor-major SBUF rows.
                psb8 = psT.tile([P, N], FP, tag="psT", name="ps_b8")
                for jh in range(2):
                    nc.tensor.matmul(
                        psb8[:, jh * 512:(jh + 1) * 512], sel8_sb[:],
                        BT_sb[:, jh * 512:(jh + 1) * 512])
                sb8 = mk(pers, (P, N), "sb8", BF)
                nc.vector.tensor_copy(sb8[:], psb8[:])
                psaccB = psT.tile([P, N], FP, tag="psT", name="ps_accB")
                for ic in range(16):
                    g, j = divmod(ic, 8)
                    th = wkT.tile([P, N], BF, tag="th", name="th")
                    nc.scalar.activation(th[:], sb8[:], AF.Tanh,
                                         bias=A_all[:, ic:ic + 1])
                    for jh in range(2):
                        nc.tensor.matmul(
                            psaccB[64 * g:64 * (g + 1),
                                   jh * 512:(jh + 1) * 512],
                            wm2g_sb[:, j, :],
                            th[:, jh * 512:(jh + 1) * 512],
                            start=(j == 0), stop=(j == 7))

                dots_sb = mk(pers, (P, N), "dots", BF)
                nc.scalar.activation(dots_sb[:], psl[:], AF.Exp,
                                     scale=rec2[:, 0:1])
                s0_sb = wkS.tile([P, 1], FP, tag="s0", name="s0")
                nc.vector.reduce_sum(s0_sb[:], dots_sb[:],
                                     axis=mybir.AxisListType.X)
                scrd = wkT.tile([P, N], BF, tag="th", name="scrd")
                dgr = wkS.tile([P, 1], FP, tag="dgr", name="dgr")
                nc.vector.scalar_tensor_tensor(
                    out=scrd[:], in0=psl[:], scalar=1.0, in1=dmask_sb[:],
                    op0=ALU.bypass, op1=ALU.mult, accum_out=dgr[:])

                # sigmoid(x) = 0.5 + 0.5*tanh(x/2): w = 0.5*(1 + wt), so
                # denom = 0.5*(S0 + sum_j dots*wt); the 0.5 is folded into
                # the host-side log (log(x/2) = log(x) - log 2)
                wt_sb = wkT.tile([P, N], BF, tag="wt", name="wt")
                nc.scalar.activation(wt_sb[:], psaccB[:], AF.Tanh,
                                     scale=0.5, bias=b2h[:])
                d1_sb = wkS.tile([P, 1], FP, tag="denom", name="denom")
                scr = wkT.tile([P, N], BF, tag="scr", name="scr")
                nc.vector.scalar_tensor_tensor(
                    out=scr[:], in0=dots_sb[:], scalar=1.0, in1=wt_sb[:],
                    op0=ALU.bypass, op1=ALU.mult, accum_out=d1_sb[:])

                outt = wkS.tile([NL, 2], FP, tag="outt", name="outt")
                nc.vector.tensor_add(outt[:, 0:1], s0_sb[:], d1_sb[:])
                nc.vector.tensor_scalar_mul(outt[:, 1:2], dgr[:],
                                            rec2[:, 0:1])
                nc.sync.dma_start(out_ext[:], outt[:])

    nc.finalize()
    return nc


_NC_CACHE = {}


def _get_nc():
    if "nc" not in _NC_CACHE:
        _NC_CACHE["nc"] = _build()
    return _NC_CACHE["nc"]


def _part3(x, p=128):
    """[(o p), f] row-major -> [p, o*f] (partition-inner layout)."""
    o = x.shape[0] // p
    return x.reshape(o, p, x.shape[1]).transpose(1, 0, 2).reshape(p, -1)


def kernel(**inputs):
    inp = {k: np.ascontiguousarray(np.asarray(v, dtype=np.float32))
           for k, v in inputs.items()}
    nc = _get_nc()

    def bf(x):
        return np.ascontiguousarray(x.astype(NPBF))

    def f8(x):
        return np.ascontiguousarray(x.astype(NPF8))

    rep = {}
    rep["fc0_w"] = f8(_part3(inp["fc0_w"]))
    rep["fc1_w"] = f8(_part3(inp["fc1_w"]))
    rep["fc2_w"] = f8(_part3(inp["fc2_w"]))
    rep["agg0_w"] = f8(_part3(inp["agg0_w"]))
    rep["agg1_w"] = f8(_part3(inp["agg1_w"]))
    rep["gcn_w1"] = f8(_part3(inp["gcn_w1"]))
    for k in ["gcn_w2", "att_w", "proj_w", "mlp1_w"]:
        rep[k] = bf(inp[k])
    rep["sel8"] = bf(
        np.kron(np.ones((1, 8), np.float32), np.eye(16, dtype=np.float32)))
    _w8 = np.kron(np.eye(8, dtype=np.float32),
                  inp["mlp2_w"].reshape(16, 1).astype(np.float32))
    _wg = np.zeros((P, 8, 64), np.float32)
    for _j in range(8):
        _wg[:, _j, 8 * _j:8 * (_j + 1)] = _w8
    rep["wm2g"] = bf(_wg.reshape(P, 8 * 64))
    rep["eye128"] = bf(np.eye(P, dtype=np.float32))
    rep["fc0_b"] = np.ascontiguousarray(
        inp["fc0_b"].reshape(HK, P).T)                     # [p, hc]
    rep["fc1_b"] = bf(inp["fc1_b"].reshape(1, H))
    rep["fc2_b"] = bf(inp["fc2_b"].reshape(1, H))
    rep["gcn_b1"] = np.ascontiguousarray(
        np.tile(inp["gcn_b1"].reshape(E), 2).reshape(P, 1))
    rep["gcn_b2"] = np.ascontiguousarray(
        np.tile(inp["gcn_b2"].reshape(E), 2).reshape(P, 1))
    rep["proj_b"] = np.ascontiguousarray(inp["proj_b"].reshape(E, 1))
    rep["attbr"] = np.ascontiguousarray(inp["att_b"].reshape(1, E))
    rep["projbr"] = np.ascontiguousarray(inp["proj_b"].reshape(1, E))
    rep["attv4"] = np.ascontiguousarray(
        np.tile(inp["att_vec"].reshape(1, E), (1, 4)))
    rep["mlp1_b"] = np.ascontiguousarray(inp["mlp1_b"].reshape(1, 16))
    rep["mlp2_b"] = np.ascontiguousarray(inp["mlp2_b"].reshape(1, 1))

    mnadj = inp["adj0"] + inp["adj1"]
    in_maps = []
    for r in range(C):
        rs = slice(r * NL, (r + 1) * NL)
        ms = slice(r * ML, (r + 1) * ML)
        d = dict(rep)
        d["feat1T"] = f8(_part3(inp["feat1"][ms].T))
        d["feat2T"] = f8(_part3(inp["feat2"][ms].T))
        d["nei0T"] = f8(_part3(inp["nei0"][:, ms].T))
        d["nei1T"] = f8(_part3(inp["nei1"][:, ms].T))
        d["recs"] = np.ascontiguousarray(np.stack(
            [1.0 / np.maximum(inp[f"nei{v}"][rs].sum(1), 1.0)
             for v in range(2)], axis=1).astype(np.float32))
        d["dmask"] = f8(np.ascontiguousarray(
            np.eye(N, dtype=np.float32)[rs]))
        d["f0m"] = f8(_part3(np.concatenate(
            [inp["feat0"][rs].T, inp["mask_feat"][rs].T], axis=1)))
        d["adj0T"] = f8(_part3(inp["adj0"][rs].T))
        d["adj1T"] = f8(_part3(inp["adj1"][rs].T))
        d["madj0T"] = f8(_part3(inp["madj0"][rs].T))
        d["madj1T"] = f8(_part3(inp["madj1"][rs].T))
        d["mnadjTf"] = f8(_part3(mnadj.T))
        in_maps.append(d)

    trace = bool(int(os.environ.get("KERNEL_TRACE", "0")))
    res = run_bass_kernel_spmd(
        nc, in_maps, core_ids=list(range(C)), trace=trace)
    if trace:
        _NC_CACHE["exec_time_ns"] = res.exec_time_ns
        _NC_CACHE["trace"] = res.instructions_and_trace
    total = 0.0
    for r in range(C):
        o = np.asarray(res.results[r]["out"], dtype=np.float64)
        total += float(np.sum(np.log(o[:, 0]) - np.log(2.0) - o[:, 1]))
    return np.float32(total / N)

